# revision 12
# baseline (speedup 1.0000x reference)
"""Trainium2 Bass kernel for a two-branch cross-attention block.

Math (per branch pair):
    x1n = LN(x1); x2n = LN(x2)
    q1,k1,v1 = split(x1n @ w_qkv1); q2,k2,v2 = split(x2n @ w_qkv2)
    out1 = softmax(q1 k2^T * s) v2 @ w_out1 + b_out1
    out2 = softmax(q2 k1^T * s) v1 @ w_out2 + b_out2

Sharding: 8 cores = 4 batches x 2 head-groups (8 heads each). Each core
handles both branches for its (batch, head-group); the out-projection
contracts over heads, so each core produces a partial [2048, 1024] per
branch (fp16) and the host sums the two head-group partials + bias.

Host folds: LN affine into W (W' = diag(g) W), softmax scale into wq,
q-bias = b_ln @ wq (added on-device per qT partition). The k-bias is
softmax-invariant (adds a per-query constant to all logits) and is
dropped; the v-bias contributes exactly (b_ln @ wv) @ w_out (softmax
weights sum to 1) and is folded into the host-side output bias.

On-device dataflow (matmul inputs bf16, fp32 accum):
    LN (DVE) -> PE transpose -> xnT [feat, tok]
    qT, kT via W stationary; v natural via xnT stationary; vA col 64 = 1
    flat head stream over (ob, h) = attn0 heads then attn1 heads:
      S^T[j,i] = kT_h^T q_h (K=64) -> exp (ACT, PSUM->SBUF bf16 es)
      AV: out[i-chunk, 65] += es[j, i-chunk]^T @ vA[j, :65]  (es is the
          stationary operand -> full 128-partition output, col 64 = Z)
      normalize: DVE divide by the Z column straight out of PSUM,
      per-head transpose into aT; out-proj accumulates 4 head-pair
      chunks; fp16 partial out.

Projection / out-projection work is emitted as "filler" chunks threaded
between attention j-steps so the PE stays dense while ACT (the exp
stream, ~2.2us per j-step vs ~1.3us of attention matmuls) is saturated.
PSUM: S double-buffer 4 banks + AV accumulator 3 banks (16 groups of
65 fp32, 7 per bank) + 1 aux bank for filler/transpose chunks.
"""

import sys
from contextlib import ExitStack

import numpy as np
import ml_dtypes

sys.path.insert(0, "/opt/trn_rl_repo")
sys.path.insert(0, "/opt/trn_rl_repo/concourse")

import concourse.bass as bass
import concourse.tile as tile
from concourse import bacc, mybir
from concourse.bass import ds, ts
from concourse.masks import make_identity

F32 = mybir.dt.float32
F16 = mybir.dt.float16
BF16 = mybir.dt.bfloat16
AF = mybir.ActivationFunctionType
ALU = mybir.AluOpType

B, N, DIM = 4, 2048, 1024
HEADS, DH = 16, 64
SCALE = DH ** -0.5
HPC = 8          # heads per core
QKCOLS = HPC * DH  # 512 qkv columns per core per tensor
TC = N // 128    # 16 token chunks
KC = DIM // 128  # 8 feature chunks
EPS = 1e-5


def _avslot(i):
    """AV psum packing: 16 i-chunk groups of 65 fp32, 7 per 512-fp32 bank."""
    b = min(i // 7, 2)
    return b, (i - 7 * b) * 65


DEBUG = False

def build_program():
    nc = bacc.Bacc(
        "TRN2",
        target_bir_lowering=False,
        debug=False,
        enable_asserts=True,
        num_devices=8,
    )
    xs, wq, wk, wv, bq, wo, outs = [], [], [], [], [], [], []
    for br in range(2):
        xs.append(nc.dram_tensor(f"x{br}", [N, DIM], BF16, kind="ExternalInput").ap())
        wq.append(nc.dram_tensor(f"wq{br}", [DIM, QKCOLS], BF16, kind="ExternalInput").ap())
        wk.append(nc.dram_tensor(f"wk{br}", [DIM, QKCOLS], BF16, kind="ExternalInput").ap())
        wv.append(nc.dram_tensor(f"wv{br}", [DIM, QKCOLS], BF16, kind="ExternalInput").ap())
        bq.append(nc.dram_tensor(f"bq{br}", [128, 4], F32, kind="ExternalInput").ap())
        wo.append(nc.dram_tensor(f"wo{br}", [QKCOLS, DIM], BF16, kind="ExternalInput").ap())
        outs.append(nc.dram_tensor(f"o{br}", [N, DIM], F16, kind="ExternalOutput").ap())
    dbg = {}
    if DEBUG:
        dbg["xnT1"] = nc.dram_tensor("d_xnT1", [128, KC * N], BF16, kind="ExternalOutput").ap()
        dbg["qT0"] = nc.dram_tensor("d_qT0", [128, 4 * N], BF16, kind="ExternalOutput").ap()
        dbg["kT1"] = nc.dram_tensor("d_kT1", [128, 4 * N], BF16, kind="ExternalOutput").ap()
        dbg["vA1"] = nc.dram_tensor("d_vA1", [128, TC * HPC * (DH + 1)], BF16, kind="ExternalOutput").ap()
        dbg["es00"] = nc.dram_tensor("d_es00", [128, N], BF16, kind="ExternalOutput").ap()
        dbg["a00"] = nc.dram_tensor("d_a00", [128, TC * DH], BF16, kind="ExternalOutput").ap()
        dbg["zr00"] = nc.dram_tensor("d_zr00", [128, TC], F32, kind="ExternalOutput").ap()
        dbg["aT0"] = nc.dram_tensor("d_aT0", [128, 4 * N], BF16, kind="ExternalOutput").ap()

    with tile.TileContext(nc) as tc:
        with ExitStack() as ctx:
            _body(ctx, tc, xs, wq, wk, wv, bq, wo, outs, dbg)
    nc.finalize()
    return nc


def _body(ctx, tc, xs, wq, wk, wv, bq, wo, outs, dbg):
    nc = tc.nc
    p_const = ctx.enter_context(tc.tile_pool(name="const", bufs=1))
    p_x = ctx.enter_context(tc.tile_pool(name="x", bufs=3))
    p_stat = ctx.enter_context(tc.tile_pool(name="stat", bufs=2))
    p_z = ctx.enter_context(tc.tile_pool(name="z", bufs=2))
    p_xnT = ctx.enter_context(tc.tile_pool(name="xnT", bufs=1))
    p_w = ctx.enter_context(tc.tile_pool(name="w", bufs=2))
    p_qk = ctx.enter_context(tc.tile_pool(name="qk", bufs=2))
    p_v = ctx.enter_context(tc.tile_pool(name="v", bufs=2))
    p_es = ctx.enter_context(tc.tile_pool(name="es", bufs=2))
    p_a = ctx.enter_context(tc.tile_pool(name="a", bufs=2))
    p_aT = ctx.enter_context(tc.tile_pool(name="aT", bufs=2))
    p_outst = ctx.enter_context(tc.tile_pool(name="outst", bufs=2))

    ident = p_const.tile([128, 128], BF16, tag="ident", name="ident")
    make_identity(nc, ident)
    bqs = []
    for br in range(2):
        bqt = p_const.tile([128, 4], F32, tag=f"bq{br}", name=f"bqt{br}")
        nc.sync.dma_start(out=bqt, in_=bq[br])
        bqs.append(bqt)
    epst = p_const.tile([128, 1], F32, tag="eps", name="epst")
    nc.vector.memset(epst, EPS)

    # ---------------- SBUF persistent tensors ----------------
    xnTs = [None, None]   # [128, KC, N] bf16, feat-major (transposed LN x)
    qTs = [None, None]    # [128, 4, N] bf16 per branch (q of that branch)
    kTs = [None, None]
    vAs = [None, None]    # [128, TC, HPC, DH+1] bf16, col DH = ones
    aTs = [None, None]    # [128, 4, N] bf16
    wv_sb = {}
    wo_sb = {}

    def phase_A(br, pps):
        """LN + transpose -> xnT[br] (two passes over re-DMA'd x)."""
        stats = p_stat.tile([128, TC, 2], F32, tag="stats", name=f"stats{br}")
        rstd = p_stat.tile([128, TC], F32, tag="rstd", name=f"rstd{br}")
        for t in range(TC):
            xt = p_x.tile([128, DIM], BF16, tag="xt", name=f"xt{br}_{t}")
            nc.sync.dma_start(out=xt, in_=xs[br][ts(t, 128), :])
            st = p_stat.tile([128, 2, 6], F32, tag="st", name=f"st{br}_{t}")
            for sg in range(2):
                nc.vector.bn_stats(out=st[:, sg, :], in_=xt[:, ts(sg, 512)])
            nc.vector.bn_aggr(out=stats[:, t, :], in_=st)
        # rstd = exp(-0.5 * ln(var + eps)), batched over all 16 tiles
        nc.scalar.activation(out=rstd, in_=stats[:, :, 1], func=AF.Ln,
                             bias=epst, scale=1.0)
        nc.scalar.activation(out=rstd, in_=rstd, func=AF.Exp, scale=-0.5)
        xnT = p_xnT.tile([128, KC, N], BF16, tag="xnT", name=f"xnT{br}")
        for t in range(TC):
            xt = p_x.tile([128, DIM], BF16, tag="xt", name=f"xt2{br}_{t}")
            nc.sync.dma_start(out=xt, in_=xs[br][ts(t, 128), :])
            zt = p_z.tile([128, DIM], BF16, tag="zt", name=f"zt{br}_{t}")
            nc.vector.tensor_scalar(out=zt, in0=xt,
                                    scalar1=stats[:, t, 0:1],
                                    scalar2=rstd[:, t:t + 1],
                                    op0=ALU.subtract, op1=ALU.mult)
            ptr = pps.tile([128, KC, 128], BF16, tag=pps._tag, name=f"ptr{br}_{t}")
            for fc in range(KC):
                nc.tensor.transpose(out=ptr[:, fc, :], in_=zt[:, ts(fc, 128)],
                                    identity=ident)
            nc.vector.tensor_copy(out=xnT[:, :, ts(t, 128)], in_=ptr)
        xnTs[br] = xnT

    # ------------- chunk emitters (prologue direct, or fillers) -------
    def qk_chunk(br, kind, wt_d, dstT, cc, quarter, psp, bias):
        """One [col-128, tok-512] chunk of a q/k projection."""
        wsb = p_w.tile([128, KC, 128], BF16, tag="w",
                       name=f"w{kind}{br}_{cc}_{quarter}")
        nc.sync.dma_start(
            out=wsb,
            in_=wt_d.rearrange("(kc p) c -> p kc c", p=128)[:, :, ts(cc, 128)])
        ps = psp.tile([128, 512], F32, tag=psp._tag,
                      name=f"ps{kind}{br}_{cc}_{quarter}")
        xnT = xnTs[br]
        for k in range(KC):
            nc.tensor.matmul(out=ps, lhsT=wsb[:, k, :],
                             rhs=xnT[:, k, ts(quarter, 512)],
                             start=(k == 0), stop=(k == KC - 1))
        if bias is not None:
            nc.vector.tensor_scalar(out=dstT[:, cc, ts(quarter, 512)], in0=ps,
                                    scalar1=bias[:, cc:cc + 1], scalar2=None,
                                    op0=ALU.add)
        else:
            nc.vector.tensor_copy(out=dstT[:, cc, ts(quarter, 512)], in_=ps)

    def v_chunk(br, j, psp):
        """One [tok-128, col-512] chunk of a v projection (natural)."""
        if br not in wv_sb:
            wsb = p_w.tile([128, KC, 512], BF16, tag="wv", bufs=1,
                           name=f"wv{br}sb")
            nc.sync.dma_start(out=wsb,
                              in_=wv[br].rearrange("(kc p) c -> p kc c", p=128))
            wv_sb[br] = wsb
        wsb = wv_sb[br]
        ps = psp.tile([128, 512], F32, tag=psp._tag, name=f"psv{br}_{j}")
        xnT = xnTs[br]
        for k in range(KC):
            nc.tensor.matmul(out=ps, lhsT=xnT[:, k, ts(j, 128)],
                             rhs=wsb[:, k, :],
                             start=(k == 0), stop=(k == KC - 1))
        nc.vector.tensor_copy(
            out=vAs[br][:, j, :, 0:DH],
            in_=ps.rearrange("p (h d) -> p h d", d=DH))

    def oproj_chunk(ob, t, cb, psp):
        """One [tok-128, dim-512] chunk of the out-projection of ob."""
        if ob not in wo_sb:
            wosb = p_w.tile([128, 4, DIM], BF16, tag="wo", bufs=1,
                            name=f"wo{ob}sb")
            nc.sync.dma_start(out=wosb,
                              in_=wo[ob].rearrange("(hd p) c -> p hd c", p=128))
            wo_sb[ob] = wosb
        wosb = wo_sb[ob]
        ps = psp.tile([128, 512], F32, tag=psp._tag, name=f"pso{ob}_{t}_{cb}")
        aT = aTs[ob]
        for hd in range(4):
            nc.tensor.matmul(out=ps, lhsT=aT[:, hd, ts(t, 128)],
                             rhs=wosb[:, hd, ts(cb, 512)],
                             start=(hd == 0), stop=(hd == 3))
        ot = p_outst.tile([128, 512], F16, tag="ot", name=f"ot{ob}_{t}_{cb}")
        nc.vector.tensor_copy(out=ot, in_=ps)
        nc.sync.dma_start(out=outs[ob][ts(t, 128), ts(cb, 512)], in_=ot)

    # ---------------- prologue ----------------
    # xnT pool has bufs=1: all xnT1 consumers (k1, v1, q1) are emitted
    # before phase_A(0) allocates xnT0; k0/v0 fillers then use xnT0 which
    # stays alive through the attention windows.
    with tc.tile_pool(name="pro_ps", bufs=2, space="PSUM") as pps:
        pps._tag = "p"
        phase_A(1, pps)
        kTs[1] = p_qk.tile([128, 4, N], BF16, tag="kT", name="kT1")
        vAs[1] = p_v.tile([128, TC, HPC, DH + 1], BF16, tag="vA", name="vA1")
        nc.vector.memset(vAs[1][:, :, :, DH:DH + 1], 1.0)
        qTs[1] = p_qk.tile([128, 4, N], BF16, tag="qT", name="qT1")
        for cc in range(4):
            for quarter in range(4):
                qk_chunk(1, "k", wk[1], kTs[1], cc, quarter, pps, None)
        for j in range(TC):
            v_chunk(1, j, pps)
        for cc in range(4):
            for quarter in range(4):
                qk_chunk(1, "q", wq[1], qTs[1], cc, quarter, pps, bqs[1])
        phase_A(0, pps)
        qTs[0] = p_qk.tile([128, 4, N], BF16, tag="qT", name="qT0")
        for quarter in range(4):
            qk_chunk(0, "q", wq[0], qTs[0], 0, quarter, pps, bqs[0])

    # ---------------- fillers for the attention windows ----------------
    kTs[0] = p_qk.tile([128, 4, N], BF16, tag="kT", name="kT0")
    vAs[0] = p_v.tile([128, TC, HPC, DH + 1], BF16, tag="vA", name="vA0")

    fillers = []

    def add_qk_fillers(br, kind, wt_d, dstT, bias, ccs):
        for cc in ccs:
            for quarter in range(4):
                fillers.append(lambda psp, br=br, kind=kind, wt_d=wt_d,
                               dstT=dstT, cc=cc, q=quarter, bias=bias:
                               qk_chunk(br, kind, wt_d, dstT, cc, q, psp, bias))

    # remaining k1 / q0 column-chunks first (attn0 heads 2+ need them),
    # then k0 / v0 for attn1.
    add_qk_fillers(0, "q", wq[0], qTs[0], bqs[0], [1, 2, 3])
    add_qk_fillers(0, "k", wk[0], kTs[0], None, [0, 1, 2, 3])

    def mset_v0(psp):
        nc.vector.memset(vAs[0][:, :, :, DH:DH + 1], 1.0)
    fillers.append(mset_v0)
    for j in range(TC):
        fillers.append(lambda psp, j=j: v_chunk(0, j, psp))

    def add_oproj_fillers(ob):
        for t in range(TC):
            for cb in range(2):
                fillers.append(lambda psp, ob=ob, t=t, cb=cb:
                               oproj_chunk(ob, t, cb, psp))

    # ---------------- attention: flat 16-head stream ----------------
    ps_s = ctx.enter_context(tc.tile_pool(name="ps_s", bufs=2, space="PSUM"))
    ps_av = ctx.enter_context(tc.tile_pool(name="ps_av", bufs=1, space="PSUM"))
    ps_aux = ctx.enter_context(tc.tile_pool(name="ps_aux", bufs=1, space="PSUM"))
    ps_aux._tag = "aux"

    def attn_head(ob, h):
        qT, kT, vA = qTs[ob], kTs[1 - ob], vAs[1 - ob]
        pt, po = h // 2, (h % 2) * 64
        pav = ps_av.tile([128, 3, 512], F32, tag="av", name=f"pav_{ob}_{h}")
        es_t = [None] * TC

        def emit_S(j):
            es = p_es.tile([128, N], BF16, tag="es", name=f"es_{ob}_{h}_{j}")
            es_t[j] = es
            for ih in range(2):
                ps = ps_s.tile([128, 1024], F32, tag="s",
                               name=f"psS_{ob}_{h}_{j}_{ih}")
                for i2 in range(2):
                    nc.tensor.matmul(
                        out=ps[:, ts(i2, 512)],
                        lhsT=kT[po:po + 64, pt, ts(j, 128)],
                        rhs=qT[po:po + 64, pt, ds(ih * 1024 + i2 * 512, 512)],
                        start=True, stop=True)
                nc.scalar.activation(out=es[:, ts(ih, 1024)], in_=ps,
                                     func=AF.Exp)

        emit_S(0)
        emit_S(1)
        for j in range(TC):
            es = es_t[j]
            for i in range(TC):
                b, off = _avslot(i)
                # HW: start=True zeroes the entire PSUM bank, so only the
                # first group of each bank may assert it (at j == 0).
                nc.tensor.matmul(out=pav[:, b, ds(off, DH + 1)],
                                 lhsT=es[:, ts(i, 128)],
                                 rhs=vA[:, j, h, :],
                                 start=(j == 0 and off == 0),
                                 stop=(j == TC - 1),
                                 skip_group_check=True)
            if DEBUG and ob == 0 and h == 0 and j == 0:
                nc.sync.dma_start(out=dbg["es00"], in_=es)
            if j + 2 < TC:
                emit_S(j + 2)
            if j % 2 == 1 and fillers:
                fillers.pop(0)(ps_aux)
        # normalize out of PSUM: a = av[:, 0:64] * (1/Z) with Z from the
        # 65th column of each group (strided gather + one reciprocal).
        a = p_a.tile([128, TC, DH], BF16, tag="atmp", name=f"a_{ob}_{h}")
        zr = p_a.tile([128, TC], F32, tag="zr", name=f"zr_{ob}_{h}")
        for b in range(3):
            n = 7 if b < 2 else 2
            nc.vector.tensor_copy(out=zr[:, b * 7:b * 7 + n],
                                  in_=pav[:, b, DH:DH + 65 * (n - 1) + 1:65])
        nc.vector.reciprocal(out=zr, in_=zr)
        for i in range(TC):
            b, off = _avslot(i)
            nc.vector.tensor_scalar(out=a[:, i, :],
                                    in0=pav[:, b, ds(off, DH)],
                                    scalar1=zr[:, i:i + 1],
                                    scalar2=None, op0=ALU.mult)
        if DEBUG and ob == 0 and h == 0:
            nc.sync.dma_start(out=dbg["a00"], in_=a.rearrange("p a b -> p (a b)"))
            nc.sync.dma_start(out=dbg["zr00"], in_=zr)
        for ib in range(4):
            ptr = ps_aux.tile([128, 4, 128], BF16, tag="aux",
                              name=f"ptrA_{ob}_{h}_{ib}")
            for q in range(4):
                i = ib * 4 + q
                nc.tensor.transpose(out=ptr[po:po + 64, q, :],
                                    in_=a[:, i, :], identity=ident)
            nc.vector.tensor_copy(out=aTs[ob][po:po + 64, pt, ts(ib, 512)],
                                  in_=ptr[po:po + 64, :, :])

    for ob in range(2):
        aTs[ob] = p_aT.tile([128, 4, N], BF16, tag="aT", name=f"aT{ob}")
        if ob == 1:
            add_oproj_fillers(0)
        for h in range(HPC):
            attn_head(ob, h)

    if DEBUG:
        nc.sync.dma_start(out=dbg["xnT1"], in_=xnTs[1].rearrange("p a b -> p (a b)"))
        nc.sync.dma_start(out=dbg["qT0"], in_=qTs[0].rearrange("p a b -> p (a b)"))
        nc.sync.dma_start(out=dbg["kT1"], in_=kTs[1].rearrange("p a b -> p (a b)"))
        nc.sync.dma_start(out=dbg["vA1"], in_=vAs[1].rearrange("p a b c -> p (a b c)"))
        nc.sync.dma_start(out=dbg["aT0"], in_=aTs[0].rearrange("p a b -> p (a b)"))

    # tail: drain remaining fillers, then out-proj of ob1
    while fillers:
        fillers.pop(0)(ps_aux)
    for t in range(TC):
        for cb in range(2):
            oproj_chunk(1, t, cb, ps_aux)


_NC = None


def _get_nc():
    global _NC
    if _NC is None:
        _NC = build_program()
    return _NC


def _make_in_maps(x1, x2, ln1_g, ln1_b, ln2_g, ln2_b,
                  w_qkv1, w_qkv2, w_out1, w_out2):
    bf16 = ml_dtypes.bfloat16
    f32 = np.float32
    branches = ((w_qkv1, ln1_g, ln1_b, w_out1), (w_qkv2, ln2_g, ln2_b, w_out2))
    per_g = []
    for g in range(2):
        cols = slice(g * QKCOLS, (g + 1) * QKCOLS)
        m = {}
        for br, (w_qkv, g_ln, b_ln, w_out) in enumerate(branches):
            wq_s = w_qkv[:, 0:DIM][:, cols]
            wk_s = w_qkv[:, DIM:2 * DIM][:, cols]
            wv_s = w_qkv[:, 2 * DIM:3 * DIM][:, cols]
            m[f"wq{br}"] = np.ascontiguousarray(
                (wq_s * g_ln[:, None] * SCALE)).astype(bf16)
            m[f"wk{br}"] = np.ascontiguousarray(wk_s * g_ln[:, None]).astype(bf16)
            m[f"wv{br}"] = np.ascontiguousarray(wv_s * g_ln[:, None]).astype(bf16)
            # q bias per column, laid out [partition 128, cc 4]
            bqv = ((b_ln @ wq_s) * SCALE).astype(f32)
            m[f"bq{br}"] = np.ascontiguousarray(bqv.reshape(4, 128).T)
            m[f"wo{br}"] = np.ascontiguousarray(w_out[cols, :]).astype(bf16)
        per_g.append(m)
    in_maps = []
    for b in range(B):
        for g in range(2):
            m = dict(per_g[g])
            m["x0"] = x1[b].astype(bf16)
            m["x1"] = x2[b].astype(bf16)
            in_maps.append(m)
    return in_maps


def run(inputs, trace=False):
    """inputs: dict as from setup_inputs(). Returns ((out1, out2), exec_time_ns)."""
    from concourse.bass_utils import run_bass_kernel_spmd

    f32 = np.float32
    ins = {k: np.asarray(v, dtype=f32) for k, v in inputs.items()}
    nc = _get_nc()
    in_maps = _make_in_maps(
        ins["x1"], ins["x2"], ins["ln1_g"], ins["ln1_b"],
        ins["ln2_g"], ins["ln2_b"], ins["w_qkv1"], ins["w_qkv2"],
        ins["w_out1"], ins["w_out2"])
    res = run_bass_kernel_spmd(nc, in_maps, core_ids=list(range(8)), trace=trace)
    r = res.results
    # v-bias contribution (softmax weights sum to 1): (b_ln @ wv) @ w_out
    bv1 = (ins["ln2_b"] @ ins["w_qkv2"][:, 2 * DIM:3 * DIM]) @ ins["w_out1"]
    bv2 = (ins["ln1_b"] @ ins["w_qkv1"][:, 2 * DIM:3 * DIM]) @ ins["w_out2"]
    out1 = np.zeros((B, N, DIM), f32)
    out2 = np.zeros((B, N, DIM), f32)
    for b in range(B):
        out1[b] = (r[2 * b]["o0"].astype(f32) + r[2 * b + 1]["o0"].astype(f32)
                   + ins["b_out1"] + bv1)
        out2[b] = (r[2 * b]["o1"].astype(f32) + r[2 * b + 1]["o1"].astype(f32)
                   + ins["b_out2"] + bv2)
    return (out1, out2), res.exec_time_ns


def kernel(**inputs):
    (out1, out2), _ = run(inputs, trace=False)
    return out1, out2


# revision 18
# speedup vs baseline: 1.4996x; 1.4996x over previous
"""Trainium2 Bass kernel for a two-branch cross-attention block.

Math (per branch pair):
    x1n = LN(x1); x2n = LN(x2)
    q1,k1,v1 = split(x1n @ w_qkv1); q2,k2,v2 = split(x2n @ w_qkv2)
    out1 = softmax(q1 k2^T * s) v2 @ w_out1 + b_out1
    out2 = softmax(q2 k1^T * s) v1 @ w_out2 + b_out2

Sharding: 8 cores = 4 batches x 2 head-groups (8 heads each). Each core
handles both branches for its (batch, head-group); the out-projection
contracts over heads, so each core produces a partial [2048, 1024] per
branch (fp16) and the host sums the two head-group partials + bias.

Host folds: LN affine into W (W' = diag(g) W), softmax scale into wq,
q-bias = b_ln @ wq (added on-device per qT partition). The k-bias is
softmax-invariant (adds a per-query constant to all logits) and is
dropped; the v-bias contributes exactly (b_ln @ wv) @ w_out (softmax
weights sum to 1) and is folded into the host-side output bias.

On-device dataflow (matmul inputs bf16, fp32 accum):
    LN (DVE) -> PE transpose -> xnT [feat, tok]
    qT, kT via W stationary; v natural via xnT stationary; vA col 64 = 1
    flat head stream over (ob, h) = attn0 heads then attn1 heads:
      S^T[j,i] = kT_h^T q_h (K=64) -> exp (ACT, PSUM->SBUF bf16 es)
      AV: out[i-chunk, 65] += es[j, i-chunk]^T @ vA[j, :65]  (es is the
          stationary operand -> full 128-partition output, col 64 = Z)
      normalize: DVE divide by the Z column straight out of PSUM,
      per-head transpose into aT; out-proj accumulates 4 head-pair
      chunks; fp16 partial out.

Projection / out-projection work is emitted as "filler" chunks threaded
between attention j-steps so the PE stays dense while ACT (the exp
stream, ~2.2us per j-step vs ~1.3us of attention matmuls) is saturated.
PSUM: S double-buffer 4 banks + AV accumulator 3 banks (16 groups of
65 fp32, 7 per bank) + 1 aux bank for filler/transpose chunks.
"""

import sys
from contextlib import ExitStack

import numpy as np
import ml_dtypes

sys.path.insert(0, "/opt/trn_rl_repo")
sys.path.insert(0, "/opt/trn_rl_repo/concourse")

import concourse.bass as bass
import concourse.tile as tile
from concourse import bacc, mybir
from concourse.bass import ds, ts
from concourse.masks import make_identity

F32 = mybir.dt.float32
F16 = mybir.dt.float16
BF16 = mybir.dt.bfloat16
AF = mybir.ActivationFunctionType
ALU = mybir.AluOpType

B, N, DIM = 4, 2048, 1024
HEADS, DH = 16, 64
SCALE = DH ** -0.5
HPC = 8          # heads per core
QKCOLS = HPC * DH  # 512 qkv columns per core per tensor
TC = N // 128    # 16 token chunks
KC = DIM // 128  # 8 feature chunks
EPS = 1e-5


def _avslot(i):
    """AV psum packing: 16 i-chunk groups of 65 fp32, 7 per 512-fp32 bank."""
    b = min(i // 7, 2)
    return b, (i - 7 * b) * 65


DEBUG = False

def build_program():
    nc = bacc.Bacc(
        "TRN2",
        target_bir_lowering=False,
        debug=False,
        enable_asserts=True,
        num_devices=8,
    )
    xs, wq, wk, wv, bq, wo, outs = [], [], [], [], [], [], []
    for br in range(2):
        xs.append(nc.dram_tensor(f"x{br}", [N, DIM], BF16, kind="ExternalInput").ap())
        wq.append(nc.dram_tensor(f"wq{br}", [DIM, QKCOLS], BF16, kind="ExternalInput").ap())
        wk.append(nc.dram_tensor(f"wk{br}", [DIM, QKCOLS], BF16, kind="ExternalInput").ap())
        wv.append(nc.dram_tensor(f"wv{br}", [DIM, QKCOLS], BF16, kind="ExternalInput").ap())
        bq.append(nc.dram_tensor(f"bq{br}", [128, 4], F32, kind="ExternalInput").ap())
        wo.append(nc.dram_tensor(f"wo{br}", [QKCOLS, DIM], BF16, kind="ExternalInput").ap())
        outs.append(nc.dram_tensor(f"o{br}", [N, DIM], F16, kind="ExternalOutput").ap())
    dbg = {}
    if DEBUG:
        dbg["xnT1"] = nc.dram_tensor("d_xnT1", [128, KC * N], BF16, kind="ExternalOutput").ap()
        dbg["qT0"] = nc.dram_tensor("d_qT0", [128, 4 * N], BF16, kind="ExternalOutput").ap()
        dbg["kT1"] = nc.dram_tensor("d_kT1", [128, 4 * N], BF16, kind="ExternalOutput").ap()
        dbg["vA1"] = nc.dram_tensor("d_vA1", [128, TC * HPC * (DH + 1)], BF16, kind="ExternalOutput").ap()
        dbg["es00"] = nc.dram_tensor("d_es00", [128, N], BF16, kind="ExternalOutput").ap()
        dbg["a00"] = nc.dram_tensor("d_a00", [128, TC * DH], BF16, kind="ExternalOutput").ap()
        dbg["zr00"] = nc.dram_tensor("d_zr00", [128, TC], F32, kind="ExternalOutput").ap()
        dbg["aT0"] = nc.dram_tensor("d_aT0", [128, 4 * N], BF16, kind="ExternalOutput").ap()

    with tile.TileContext(nc) as tc:
        with ExitStack() as ctx:
            _body(ctx, tc, xs, wq, wk, wv, bq, wo, outs, dbg)
    nc.finalize()
    return nc


def _body(ctx, tc, xs, wq, wk, wv, bq, wo, outs, dbg):
    nc = tc.nc
    p_const = ctx.enter_context(tc.tile_pool(name="const", bufs=1))
    p_x = ctx.enter_context(tc.tile_pool(name="x", bufs=3))
    p_stat = ctx.enter_context(tc.tile_pool(name="stat", bufs=2))
    p_z = ctx.enter_context(tc.tile_pool(name="z", bufs=2))
    p_xnT = ctx.enter_context(tc.tile_pool(name="xnT", bufs=1))
    p_w = ctx.enter_context(tc.tile_pool(name="w", bufs=2))
    p_qk = ctx.enter_context(tc.tile_pool(name="qk", bufs=2))
    p_v = ctx.enter_context(tc.tile_pool(name="v", bufs=2))
    p_es = ctx.enter_context(tc.tile_pool(name="es", bufs=2))
    p_a = ctx.enter_context(tc.tile_pool(name="a", bufs=2))
    p_aT = ctx.enter_context(tc.tile_pool(name="aT", bufs=2))
    p_outst = ctx.enter_context(tc.tile_pool(name="outst", bufs=2))

    ident = p_const.tile([128, 128], BF16, tag="ident", name="ident")
    make_identity(nc, ident)
    bqs = []
    for br in range(2):
        bqt = p_const.tile([128, 4], F32, tag=f"bq{br}", name=f"bqt{br}")
        nc.sync.dma_start(out=bqt, in_=bq[br])
        bqs.append(bqt)
    epst = p_const.tile([128, 1], F32, tag="eps", name="epst")
    nc.vector.memset(epst, EPS)

    # ---------------- SBUF persistent tensors ----------------
    xnTs = [None, None]   # [128, KC, N] bf16, feat-major (transposed LN x)
    qTs = [None, None]    # [128, 4, N] bf16 per branch (q of that branch)
    kTs = [None, None]
    vAs = [None, None]    # [128, TC, HPC, DH+1] bf16, col DH = ones
    aTs = [None, None]    # [128, 4, N] bf16
    wv_sb = {}
    wo_sb = {}

    def phase_A_stats(br):
        """LN stats pass; returns (stats, rstd, step, rstd_all)."""
        stats = p_stat.tile([128, TC, 2], F32, tag="stats", name=f"stats{br}")
        rstd = p_stat.tile([128, TC], F32, tag="rstd", name=f"rstd{br}")

        def step(t):
            xt = p_x.tile([128, DIM], BF16, tag="xt", name=f"xt{br}_{t}")
            nc.sync.dma_start(out=xt, in_=xs[br][ts(t, 128), :])
            st = p_stat.tile([128, 2, 6], F32, tag="st", name=f"st{br}_{t}")
            for sg in range(2):
                nc.vector.bn_stats(out=st[:, sg, :], in_=xt[:, ts(sg, 512)])
            nc.vector.bn_aggr(out=stats[:, t, :], in_=st)

        def rstd_all():
            # rstd = exp(-0.5 * ln(var + eps)), batched over all 16 tiles
            nc.scalar.activation(out=rstd, in_=stats[:, :, 1], func=AF.Ln,
                                 bias=epst, scale=1.0)
            nc.scalar.activation(out=rstd, in_=rstd, func=AF.Exp, scale=-0.5)
        return stats, rstd, step, rstd_all

    def phase_A_z(br, stats, rstd, pps):
        xnT = p_xnT.tile([128, KC, N], BF16, tag="xnT", name=f"xnT{br}")
        for t in range(TC):
            xt = p_x.tile([128, DIM], BF16, tag="xt", name=f"xt2{br}_{t}")
            nc.sync.dma_start(out=xt, in_=xs[br][ts(t, 128), :])
            zt = p_z.tile([128, DIM], BF16, tag="zt", name=f"zt{br}_{t}")
            nc.vector.tensor_scalar(out=zt, in0=xt,
                                    scalar1=stats[:, t, 0:1],
                                    scalar2=rstd[:, t:t + 1],
                                    op0=ALU.subtract, op1=ALU.mult)
            ptr = pps.tile([128, KC, 128], BF16, tag=pps._tag, name=f"ptr{br}_{t}")
            for fc in range(KC):
                nc.tensor.transpose(out=ptr[:, fc, :], in_=zt[:, ts(fc, 128)],
                                    identity=ident)
            nc.vector.tensor_copy(out=xnT[:, :, ts(t, 128)], in_=ptr)
        xnTs[br] = xnT

    # ------------- chunk emitters (prologue direct, or fillers) -------
    def qk_chunk(br, kind, wt_d, dstT, cc, quarter, psp, bias):
        """One [col-128, tok-512] chunk of a q/k projection."""
        wsb = p_w.tile([128, KC, 128], BF16, tag="w",
                       name=f"w{kind}{br}_{cc}_{quarter}")
        nc.sync.dma_start(
            out=wsb,
            in_=wt_d.rearrange("(kc p) c -> p kc c", p=128)[:, :, ts(cc, 128)])
        ps = psp.tile([128, 512], F32, tag=psp._tag,
                      name=f"ps{kind}{br}_{cc}_{quarter}")
        xnT = xnTs[br]
        for k in range(KC):
            nc.tensor.matmul(out=ps, lhsT=wsb[:, k, :],
                             rhs=xnT[:, k, ts(quarter, 512)],
                             start=(k == 0), stop=(k == KC - 1))
        if bias is not None:
            nc.vector.tensor_scalar(out=dstT[:, cc, ts(quarter, 512)], in0=ps,
                                    scalar1=bias[:, cc:cc + 1], scalar2=None,
                                    op0=ALU.add)
        else:
            nc.vector.tensor_copy(out=dstT[:, cc, ts(quarter, 512)], in_=ps)

    def v_chunk(br, j, psp):
        """One [tok-128, col-512] chunk of a v projection (natural)."""
        if br not in wv_sb:
            wsb = p_w.tile([128, KC, 512], BF16, tag="wv", bufs=1,
                           name=f"wv{br}sb")
            nc.sync.dma_start(out=wsb,
                              in_=wv[br].rearrange("(kc p) c -> p kc c", p=128))
            wv_sb[br] = wsb
        wsb = wv_sb[br]
        ps = psp.tile([128, 512], F32, tag=psp._tag, name=f"psv{br}_{j}")
        xnT = xnTs[br]
        for k in range(KC):
            nc.tensor.matmul(out=ps, lhsT=xnT[:, k, ts(j, 128)],
                             rhs=wsb[:, k, :],
                             start=(k == 0), stop=(k == KC - 1))
        nc.vector.tensor_copy(
            out=vAs[br][:, j, :, 0:DH],
            in_=ps.rearrange("p (h d) -> p h d", d=DH))

    def oproj_chunk(ob, t, cb, psp):
        """One [tok-128, dim-512] chunk of the out-projection of ob."""
        if ob not in wo_sb:
            wosb = p_w.tile([128, 4, DIM], BF16, tag="wo", bufs=1,
                            name=f"wo{ob}sb")
            nc.sync.dma_start(out=wosb,
                              in_=wo[ob].rearrange("(hd p) c -> p hd c", p=128))
            wo_sb[ob] = wosb
        wosb = wo_sb[ob]
        ps = psp.tile([128, 512], F32, tag=psp._tag, name=f"pso{ob}_{t}_{cb}")
        aT = aTs[ob]
        for hd in range(4):
            nc.tensor.matmul(out=ps, lhsT=aT[:, hd, ts(t, 128)],
                             rhs=wosb[:, hd, ts(cb, 512)],
                             start=(hd == 0), stop=(hd == 3))
        ot = p_outst.tile([128, 512], F16, tag="ot", name=f"ot{ob}_{t}_{cb}")
        nc.vector.tensor_copy(out=ot, in_=ps)
        nc.sync.dma_start(out=outs[ob][ts(t, 128), ts(cb, 512)], in_=ot)

    # ---------------- prologue ----------------
    # xnT pool has bufs=1: all xnT1 consumers (k1, v1, q1) are emitted
    # before phase_A(0) allocates xnT0; k0/v0 fillers then use xnT0 which
    # stays alive through the attention windows.
    with tc.tile_pool(name="pro_ps", bufs=2, space="PSUM") as pps:
        pps._tag = "p"
        s1, r1, step1, rstd1 = phase_A_stats(1)
        for t in range(TC):
            step1(t)
        rstd1()
        phase_A_z(1, s1, r1, pps)
        kTs[1] = p_qk.tile([128, 4, N], BF16, tag="kT", name="kT1")
        vAs[1] = p_v.tile([128, TC, HPC, DH + 1], BF16, tag="vA", name="vA1")
        nc.vector.memset(vAs[1][:, :, :, DH:DH + 1], 1.0)
        qTs[1] = p_qk.tile([128, 4, N], BF16, tag="qT", name="qT1")
        chunks = []
        for cc in range(4):
            for quarter in range(4):
                chunks.append(lambda cc=cc, q=quarter:
                              qk_chunk(1, "k", wk[1], kTs[1], cc, q, pps, None))
        for j in range(TC):
            chunks.append(lambda j=j: v_chunk(1, j, pps))
        for cc in range(4):
            for quarter in range(4):
                chunks.append(lambda cc=cc, q=quarter:
                              qk_chunk(1, "q", wq[1], qTs[1], cc, q, pps, bqs[1]))
        # branch-0 LN stats ride the DVE between projection copies
        s0, r0, step0, rstd0 = phase_A_stats(0)
        si = 0
        for i, ch in enumerate(chunks):
            ch()
            if i % 3 == 0 and si < TC:
                step0(si)
                si += 1
        while si < TC:
            step0(si)
            si += 1
        rstd0()
        phase_A_z(0, s0, r0, pps)
        qTs[0] = p_qk.tile([128, 4, N], BF16, tag="qT", name="qT0")
        for quarter in range(4):
            qk_chunk(0, "q", wq[0], qTs[0], 0, quarter, pps, bqs[0])

    # ---------------- fillers for the attention windows ----------------
    kTs[0] = p_qk.tile([128, 4, N], BF16, tag="kT", name="kT0")
    vAs[0] = p_v.tile([128, TC, HPC, DH + 1], BF16, tag="vA", name="vA0")

    fillers = []

    def add_qk_fillers(br, kind, wt_d, dstT, bias, ccs):
        for cc in ccs:
            for quarter in range(4):
                fillers.append(lambda psp, br=br, kind=kind, wt_d=wt_d,
                               dstT=dstT, cc=cc, q=quarter, bias=bias:
                               qk_chunk(br, kind, wt_d, dstT, cc, q, psp, bias))

    # remaining k1 / q0 column-chunks first (attn0 heads 2+ need them),
    # then k0 / v0 for attn1.
    add_qk_fillers(0, "q", wq[0], qTs[0], bqs[0], [1, 2, 3])
    add_qk_fillers(0, "k", wk[0], kTs[0], None, [0, 1, 2, 3])

    def mset_v0(psp):
        nc.vector.memset(vAs[0][:, :, :, DH:DH + 1], 1.0)
    fillers.append(mset_v0)
    for j in range(TC):
        fillers.append(lambda psp, j=j: v_chunk(0, j, psp))

    def add_oproj_fillers(ob):
        for t in range(TC):
            for cb in range(2):
                fillers.append(lambda psp, ob=ob, t=t, cb=cb:
                               oproj_chunk(ob, t, cb, psp))

    # ---------------- attention: flat 16-head stream ----------------
    ps_s = ctx.enter_context(tc.tile_pool(name="ps_s", bufs=2, space="PSUM"))
    ps_av = ctx.enter_context(tc.tile_pool(name="ps_av", bufs=1, space="PSUM"))
    ps_aux = ctx.enter_context(tc.tile_pool(name="ps_aux", bufs=1, space="PSUM"))
    ps_aux._tag = "aux"

    def make_head(ob, h):
        """Head descriptor with emit_S / j_loop / finalize closures."""
        qT, kT, vA = qTs[ob], kTs[1 - ob], vAs[1 - ob]
        pt, po = h // 2, (h % 2) * 64
        es_t = [None] * TC
        st = {"pav": None}

        def emit_S(j):
            es = p_es.tile([128, N], BF16, tag="es", name=f"es_{ob}_{h}_{j}")
            es_t[j] = es
            for ih in range(2):
                ps = ps_s.tile([128, 1024], F32, tag="s",
                               name=f"psS_{ob}_{h}_{j}_{ih}")
                for i2 in range(2):
                    nc.tensor.matmul(
                        out=ps[:, ts(i2, 512)],
                        lhsT=kT[po:po + 64, pt, ts(j, 128)],
                        rhs=qT[po:po + 64, pt, ds(ih * 1024 + i2 * 512, 512)],
                        start=True, stop=True)
                nc.scalar.activation(out=es[:, ts(ih, 1024)], in_=ps,
                                     func=AF.Exp)

        def j_loop(next_S):
            pav = ps_av.tile([128, 3, 512], F32, tag="av", name=f"pav_{ob}_{h}")
            st["pav"] = pav
            for j in range(TC):
                es = es_t[j]
                for i in range(TC):
                    b, off = _avslot(i)
                    # HW: start=True zeroes the entire PSUM bank, so only
                    # the first group of each bank may assert it (at j == 0).
                    nc.tensor.matmul(out=pav[:, b, ds(off, DH + 1)],
                                     lhsT=es[:, ts(i, 128)],
                                     rhs=vA[:, j, h, :],
                                     start=(j == 0 and off == 0),
                                     stop=(j == TC - 1),
                                     skip_group_check=True)
                if j + 2 < TC:
                    emit_S(j + 2)
                elif next_S is not None:
                    # prefetch the NEXT head's first S batches so its exp
                    # stream is never serialized behind our last AV
                    next_S(j - (TC - 2))
                if j % 2 == 1 and fillers:
                    fillers.pop(0)(ps_aux)

        def finalize():
            pav = st["pav"]
            # normalize out of PSUM: a = av[:, 0:64] * (1/Z) with Z from the
            # 65th column of each group (strided gather + one reciprocal).
            a = p_a.tile([128, TC, DH], BF16, tag="atmp", name=f"a_{ob}_{h}")
            zr = p_a.tile([128, TC], F32, tag="zr", name=f"zr_{ob}_{h}")
            for b in range(3):
                n = 7 if b < 2 else 2
                nc.vector.tensor_copy(out=zr[:, b * 7:b * 7 + n],
                                      in_=pav[:, b, DH:DH + 65 * (n - 1) + 1:65])
            nc.vector.reciprocal(out=zr, in_=zr)
            for i in range(TC):
                b, off = _avslot(i)
                nc.vector.tensor_scalar(out=a[:, i, :],
                                        in0=pav[:, b, ds(off, DH)],
                                        scalar1=zr[:, i:i + 1],
                                        scalar2=None, op0=ALU.mult)
            for ib in range(4):
                ptr = ps_aux.tile([128, 4, 128], BF16, tag="aux",
                                  name=f"ptrA_{ob}_{h}_{ib}")
                for q in range(4):
                    i = ib * 4 + q
                    nc.tensor.transpose(out=ptr[po:po + 64, q, :],
                                        in_=a[:, i, :], identity=ident)
                nc.vector.tensor_copy(out=aTs[ob][po:po + 64, pt, ts(ib, 512)],
                                      in_=ptr[po:po + 64, :, :])
        return emit_S, j_loop, finalize

    aTs[0] = p_aT.tile([128, 4, N], BF16, tag="aT", name="aT0")
    aTs[1] = p_aT.tile([128, 4, N], BF16, tag="aT", name="aT1")
    heads = [make_head(ob, h) for ob in range(2) for h in range(HPC)]
    pending = None
    for idx, (emit_S, j_loop, finalize) in enumerate(heads):
        if idx == 0:
            emit_S(0)
            emit_S(1)
        if pending is not None:
            pending()
        if idx == HPC:
            add_oproj_fillers(0)
        nxt = heads[idx + 1][0] if idx + 1 < len(heads) else None
        j_loop(nxt)
        pending = heads[idx][2]
    pending()

    if DEBUG:
        nc.sync.dma_start(out=dbg["xnT1"], in_=xnTs[1].rearrange("p a b -> p (a b)"))
        nc.sync.dma_start(out=dbg["qT0"], in_=qTs[0].rearrange("p a b -> p (a b)"))
        nc.sync.dma_start(out=dbg["kT1"], in_=kTs[1].rearrange("p a b -> p (a b)"))
        nc.sync.dma_start(out=dbg["vA1"], in_=vAs[1].rearrange("p a b c -> p (a b c)"))
        nc.sync.dma_start(out=dbg["aT0"], in_=aTs[0].rearrange("p a b -> p (a b)"))

    # tail: drain remaining fillers, then out-proj of ob1 (double-
    # buffered through the now-free S psum pool)
    while fillers:
        fillers.pop(0)(ps_aux)
    wosb = p_w.tile([128, 4, DIM], BF16, tag="wo", bufs=1, name="wo1sb")
    nc.sync.dma_start(out=wosb,
                      in_=wo[1].rearrange("(hd p) c -> p hd c", p=128))
    for t in range(TC):
        ps = ps_s.tile([128, 1024], F32, tag="s", name=f"psoT_{t}")
        for cb in range(2):
            for hd in range(4):
                nc.tensor.matmul(out=ps[:, ts(cb, 512)],
                                 lhsT=aTs[1][:, hd, ts(t, 128)],
                                 rhs=wosb[:, hd, ts(cb, 512)],
                                 start=(hd == 0), stop=(hd == 3))
        ot = p_outst.tile([128, 1024], F16, tag="ot", name=f"otT_{t}")
        nc.vector.tensor_copy(out=ot, in_=ps)
        nc.sync.dma_start(out=outs[1][ts(t, 128), :], in_=ot)


_NC = None


def _get_nc():
    global _NC
    if _NC is None:
        _NC = build_program()
    return _NC


def _make_in_maps(x1, x2, ln1_g, ln1_b, ln2_g, ln2_b,
                  w_qkv1, w_qkv2, w_out1, w_out2):
    bf16 = ml_dtypes.bfloat16
    f32 = np.float32
    branches = ((w_qkv1, ln1_g, ln1_b, w_out1), (w_qkv2, ln2_g, ln2_b, w_out2))
    per_g = []
    for g in range(2):
        cols = slice(g * QKCOLS, (g + 1) * QKCOLS)
        m = {}
        for br, (w_qkv, g_ln, b_ln, w_out) in enumerate(branches):
            wq_s = w_qkv[:, 0:DIM][:, cols]
            wk_s = w_qkv[:, DIM:2 * DIM][:, cols]
            wv_s = w_qkv[:, 2 * DIM:3 * DIM][:, cols]
            m[f"wq{br}"] = np.ascontiguousarray(
                (wq_s * g_ln[:, None] * SCALE)).astype(bf16)
            m[f"wk{br}"] = np.ascontiguousarray(wk_s * g_ln[:, None]).astype(bf16)
            m[f"wv{br}"] = np.ascontiguousarray(wv_s * g_ln[:, None]).astype(bf16)
            # q bias per column, laid out [partition 128, cc 4]
            bqv = ((b_ln @ wq_s) * SCALE).astype(f32)
            m[f"bq{br}"] = np.ascontiguousarray(bqv.reshape(4, 128).T)
            m[f"wo{br}"] = np.ascontiguousarray(w_out[cols, :]).astype(bf16)
        per_g.append(m)
    in_maps = []
    for b in range(B):
        for g in range(2):
            m = dict(per_g[g])
            m["x0"] = x1[b].astype(bf16)
            m["x1"] = x2[b].astype(bf16)
            in_maps.append(m)
    return in_maps


def run(inputs, trace=False):
    """inputs: dict as from setup_inputs(). Returns ((out1, out2), exec_time_ns)."""
    from concourse.bass_utils import run_bass_kernel_spmd

    f32 = np.float32
    ins = {k: np.asarray(v, dtype=f32) for k, v in inputs.items()}
    nc = _get_nc()
    in_maps = _make_in_maps(
        ins["x1"], ins["x2"], ins["ln1_g"], ins["ln1_b"],
        ins["ln2_g"], ins["ln2_b"], ins["w_qkv1"], ins["w_qkv2"],
        ins["w_out1"], ins["w_out2"])
    res = run_bass_kernel_spmd(nc, in_maps, core_ids=list(range(8)), trace=trace)
    r = res.results
    # v-bias contribution (softmax weights sum to 1): (b_ln @ wv) @ w_out
    bv1 = (ins["ln2_b"] @ ins["w_qkv2"][:, 2 * DIM:3 * DIM]) @ ins["w_out1"]
    bv2 = (ins["ln1_b"] @ ins["w_qkv1"][:, 2 * DIM:3 * DIM]) @ ins["w_out2"]
    out1 = np.zeros((B, N, DIM), f32)
    out2 = np.zeros((B, N, DIM), f32)
    for b in range(B):
        out1[b] = (r[2 * b]["o0"].astype(f32) + r[2 * b + 1]["o0"].astype(f32)
                   + ins["b_out1"] + bv1)
        out2[b] = (r[2 * b]["o1"].astype(f32) + r[2 * b + 1]["o1"].astype(f32)
                   + ins["b_out2"] + bv2)
    return (out1, out2), res.exec_time_ns


def kernel(**inputs):
    (out1, out2), _ = run(inputs, trace=False)
    return out1, out2
